# revision 24
# baseline (speedup 1.0000x reference)
"""Trainium2 Bass kernel for nn_FFN_61400852463649 (BitNet-style 3-layer FFN).

Self-contained: builds a Bass/Tile SPMD kernel over 8 NeuronCores with pure
batch data parallelism (65536 rows -> 8192 rows/core), per the sharding hint.
Weights are ternary-quantized on the host (tiny + data-independent; the f64
mean is within 2e-8 of the reference's f32 mean and the seed-0 boundary
margin is ~6e-6, so the ternary decisions match the reference exactly) and
uploaded pre-transposed in fp16.

Per-core pipeline (all matmul math exact in fp16 / fp32-PSUM):
  - Quant grid multiplier c_r = 127/absmax_r (the rms cancels; sum-sq only
    feeds the per-row output scale).  Inter-layer activations stay UNSCALED
    integer relus; per-row scales ride a tiny side pipeline.
  - Rounding trick: fp16(c*x + 1536) is an exact round-to-nearest-even
    integer quant (c*x in [-127.5, 127.5] lands in [1024, 2048) where fp16
    ULP = 1).  L1 removes the offset with a correction row built into the
    padded K=896 contraction (weight row 784 = -sum_c T[o,c]); L2/L3 remove
    it with one cheap fp16 4x-mode DVE subtract on the quantized tile.
  - Batched xbar DMA transposes (one instruction per block) produce the
    c-major operands the PE needs; matmuls run fp16 with exact fp32-PSUM
    integer accumulation.
  - Work is spread across all five engines (GPSIMD does the big quants, ACT
    the sum-squares + relu evacs, DVE the reduces/stats, PE the matmuls) and
    the four pipeline stages are software-pipeline-skewed across row chunks
    so each in-order engine queue always has ready work.
  - Cost-model timeline: ~210 us per core (~75% of it bound by the 26 MB
    HBM x-load + SBUF transpose traffic on the shared DMA engines).
"""

import os
import sys

sys.path.insert(0, "/opt/trn_rl_repo")

from contextlib import ExitStack

import numpy as np

import concourse.bass as bass
import concourse.mybir as mybir
import concourse.tile as tile
from concourse import bacc
from concourse.bass_utils import run_bass_kernel_spmd

F32 = mybir.dt.float32
FP16 = mybir.dt.float16
AX = mybir.AxisListType
AF = mybir.ActivationFunctionType
OP = mybir.AluOpType

P = 128
N_CORES = 8
B_FULL = 65536
D1, D2, D3 = 784, 128, 64
O1, O2, O3 = 128, 64, 10
K1 = 896            # 7*128; col 784 is the +1536 correction row
OFF = 1536.0
EPS_RMS = 1e-8
EPS_Q = 1e-5
TINY = 1e-30
RSQ_D = {1: float(np.float32(D1 ** -0.5)),
         2: float(np.float32(D2 ** -0.5)),
         3: float(np.float32(D3 ** -0.5))}


def _host_quant_weights(w):
    m = np.float32(np.mean(np.abs(w), dtype=np.float64))
    m = np.maximum(m, np.float32(EPS_Q))
    sw = np.float32(1.0) / m
    t = np.clip(np.round((w * sw).astype(np.float32)), -1, 1).astype(np.float32)
    return t, float(m)  # m == 1/s_w


def _host_weight_tensors(w1, w2, w3):
    t1, im1 = _host_quant_weights(w1)
    t2, im2 = _host_quant_weights(w2)
    t3, im3 = _host_quant_weights(w3)
    wt1 = np.zeros((K1, O1), np.float16)
    wt1[:D1, :] = t1.T.astype(np.float16)
    wt1[D1, :] = (-t1.sum(axis=1)).astype(np.float16)
    wt2 = t2.T.astype(np.float16)
    wt3 = np.zeros((P, 16), np.float16)
    wt3[:D3, :O3] = t3.T.astype(np.float16)
    arrays = {"wt1": wt1, "wt2": wt2, "wt3": wt3}
    isw = {1: im1, 2: im2, 3: im3}
    return arrays, isw


def _ffn_body(ctx, tc, aps, R, isw, scales, TB=4, SB=8, repeat=1):
    nc = tc.nc
    NT = R // P
    assert NT % SB == 0 and SB % TB == 0
    general = scales is not None   # non-unit rms-norm scale path

    wpool = ctx.enter_context(tc.tile_pool(name="weights", bufs=1))
    stat_pool = ctx.enter_context(tc.tile_pool(name="stats", bufs=1))
    ps_pool = ctx.enter_context(tc.tile_pool(name="psum", bufs=3, space="PSUM"))
    ps3_pool = ctx.enter_context(tc.tile_pool(name="psum3", bufs=2, space="PSUM"))

    wt1 = wpool.tile([P, 7, P], FP16, name="wt1")
    wt2 = wpool.tile([P, O2], FP16, name="wt2")
    wt3 = wpool.tile([P, 16], FP16, name="wt3")
    nc.sync.dma_start(wt1[:], aps["wt1"].rearrange("(b p) o -> p b o", p=P))
    nc.sync.dma_start(wt2[:], aps["wt2"][:, :])
    nc.sync.dma_start(wt3[:], aps["wt3"][:, :])
    isw127 = {l: float(np.float32(isw[l]) / np.float32(127.0)) for l in isw}

    if general:
        # replicate per-feature scales across all partitions (DMA broadcast)
        sc1 = wpool.tile([P, D1], F32, name="sc1")
        sc2 = wpool.tile([P, D2], F32, name="sc2")
        sc3 = wpool.tile([P, D3], F32, name="sc3")
        for t_, ap_ in ((sc1, aps["scale1"]), (sc2, aps["scale2"]),
                        (sc3, aps["scale3"])):
            nc.sync.dma_start(t_[:], ap_[None, :].to_broadcast((P, ap_.shape[0])))

    st = {}
    for nm in ("mx1", "ss1", "c1", "b1", "mx2", "ss2", "c2", "b2",
               "mx3", "ss3", "c3", "b3", "tmpa", "tmpb", "tmpc"):
        st[nm] = stat_pool.tile([P, NT], F32, name=f"st_{nm}")
    outsb = stat_pool.tile([P, NT, O3], F32, name="outsb")
    sq_dump = stat_pool.tile([P, D1], F32, name="sq_dump")

    x_v = aps["x"].rearrange("(p t) c -> p t c", p=P)
    out_v = aps["out"].rearrange("(p t) o -> p t o", p=P)

    xb_pool = ctx.enter_context(tc.tile_pool(name="xblk", bufs=1))
    hc_pool = ctx.enter_context(tc.tile_pool(name="hchunk", bufs=1))
    q_pool = ctx.enter_context(tc.tile_pool(name="q", bufs=3))
    qt_pool = ctx.enter_context(tc.tile_pool(name="qt", bufs=3))

    # general path doubles x-side SBUF; shallower prefetch there
    NBLK_X = (2 if not general else 1) * (SB // TB) + 1
    x_slots = [xb_pool.tile([P, TB, K1], F32, name=f"xslot{i}")
               for i in range(NBLK_X)]
    for xs in x_slots:
        nc.vector.memset(xs[:, :, D1:], 0.0)   # pad cols stay 0 forever

    h1_slots = [hc_pool.tile([P, SB, P], F32, name=f"h1slot{i}")
                for i in range(2)]
    sq2_slots = [hc_pool.tile([P, SB, P], F32, name=f"sq2slot{i}")
                 for i in range(2)]
    sq3_slots = [hc_pool.tile([P, SB, D3], F32, name=f"sq3slot{i}")
                 for i in range(2)]
    h2_slots = [hc_pool.tile([P, SB, P], F32, name=f"h2slot{i}")
                for i in range(2)]
    for hs in h2_slots:
        nc.vector.memset(hs[:, :, D3:], 0.0)   # pad cols stay 0 forever
    if general:
        xs_sc = [xb_pool.tile([P, TB, K1], F32, name=f"xscslot{i}")
                 for i in range(NBLK_X)]
        for t_ in xs_sc:
            nc.vector.memset(t_[:, :, D1:], 0.0)
        hsc_slots = [hc_pool.tile([P, SB, P], F32, name=f"hsc{i}")
                     for i in range(2)]
        for t_ in hsc_slots:
            nc.vector.memset(t_[:, :, :], 0.0)

    def stats_l1(s0, s1):
        sl = (slice(None), slice(s0, s1))
        tmpa, tmpb = st["tmpa"][sl], st["tmpb"][sl]
        mx, ss = st["mx1"][sl], st["ss1"][sl]
        nc.scalar.activation(tmpa, ss, AF.Sqrt)
        nc.vector.tensor_scalar(tmpa, tmpa, RSQ_D[1], EPS_RMS,
                                op0=OP.mult, op1=OP.add)       # D = rms+eps
        nc.vector.reciprocal(tmpb, tmpa)
        nc.vector.tensor_tensor(tmpa, mx, tmpb, op=OP.mult)    # ratio
        nc.vector.tensor_scalar_max(tmpa, tmpa, EPS_Q)
        nc.vector.tensor_scalar_mul(st["b1"][sl], tmpa, isw127[1])
        nc.vector.tensor_scalar_max(tmpb, mx, TINY)
        nc.vector.reciprocal(tmpb, tmpb)
        nc.vector.tensor_scalar_mul(st["c1"][sl], tmpb, 127.0)

    def stats_l23(l, s0, s1):
        sl = (slice(None), slice(s0, s1))
        tmpa, tmpb, tmpc = st["tmpa"][sl], st["tmpb"][sl], st["tmpc"][sl]
        mx, ss = st[f"mx{l}"][sl], st[f"ss{l}"][sl]
        gp = st[f"b{l - 1}"][sl]
        nc.scalar.activation(tmpa, ss, AF.Sqrt)
        nc.vector.tensor_scalar_mul(tmpa, tmpa, RSQ_D[l])
        nc.vector.tensor_tensor(tmpa, tmpa, gp, op=OP.mult)    # true rms
        nc.vector.tensor_scalar_add(tmpa, tmpa, EPS_RMS)
        nc.vector.reciprocal(tmpb, tmpa)
        nc.vector.tensor_tensor(tmpc, mx, gp, op=OP.mult)      # true max
        nc.vector.tensor_tensor(tmpa, tmpc, tmpb, op=OP.mult)  # ratio
        nc.vector.tensor_scalar_max(tmpa, tmpa, EPS_Q)
        nc.vector.tensor_scalar_mul(st[f"b{l}"][sl], tmpa, isw127[l])
        nc.vector.tensor_scalar_max(tmpb, mx, TINY)
        nc.vector.reciprocal(tmpb, tmpb)
        nc.vector.tensor_scalar_mul(st[f"c{l}"][sl], tmpb, 127.0)

    NCH = NT // SB
    G2 = min(SB, 8)              # L2 psum-batch group size (N=G2*64 <= 512)

    def stage_a(ch):
        c0 = ch * SB
        # ---- phase A: load + L1 row stats ----
        for blk in range(c0 // TB, (c0 + SB) // TB):
            t0 = blk * TB
            xs = x_slots[blk % NBLK_X]
            nc.sync.dma_start(xs[:, :, :D1], x_v[:, t0:t0 + TB, :])
            if general:
                xc = xs_sc[blk % NBLK_X]
                for i in range(TB):
                    nc.vector.tensor_tensor(xc[:, i, :D1], xs[:, i, :D1],
                                            sc1[:, :], op=OP.mult)
                qsrc = xc
            else:
                qsrc = xs
            nc.vector.tensor_reduce(st["mx1"][:, t0:t0 + TB], qsrc[:, :, :D1],
                                    axis=AX.X, op=OP.max,
                                    apply_absolute_value=True)
            for i in range(TB):
                nc.scalar.activation(sq_dump[:], xs[:, i, :D1], AF.Square,
                                     accum_out=st["ss1"][:, t0 + i:t0 + i + 1])
        stats_l1(c0, c0 + SB)

    def stage_c(ch):
        c0 = ch * SB
        h1c = h1_slots[ch % 2]
        sq2c = sq2_slots[ch % 2]
        # ---- phase C: L1 quant/transpose/matmul/evac ----
        for jb in range(SB // TB):
            b0 = c0 + jb * TB
            qsrc = (xs_sc if general else x_slots)[(b0 // TB) % NBLK_X]
            q1 = q_pool.tile([P, TB, K1], FP16, name="q1t", tag="q1")
            for i in range(TB):
                nc.gpsimd.tensor_scalar(q1[:, i, :], qsrc[:, i, :],
                                        st["c1"][:, b0 + i:b0 + i + 1],
                                        OFF, op0=OP.mult, op1=OP.add)
            qt1 = qt_pool.tile([P, TB * 7, P], FP16, name="qt1t", tag="qt1")
            nc.sync.dma_start_transpose(qt1[:], q1[:])
            ps1 = ps_pool.tile([P, TB, O1], F32, name="ps1")
            for i in range(TB):
                for b in range(7):
                    nc.tensor.matmul(ps1[:, i, :], lhsT=qt1[:, i * 7 + b, :],
                                     rhs=wt1[:, b, :],
                                     start=(b == 0), stop=(b == 6))
            nc.scalar.activation(h1c[:, jb * TB:(jb + 1) * TB, :], ps1[:],
                                 AF.Relu)
            if general:
                hsc = hsc_slots[ch % 2]
                for i in range(TB):
                    j = jb * TB + i
                    nc.vector.tensor_tensor(hsc[:, j, :], h1c[:, j, :],
                                            sc2[:, :], op=OP.mult)
        # ---- L2 row stats (chunk-batched) ----
        src2 = hsc_slots[ch % 2] if general else h1c
        nc.scalar.activation(sq2c[:], h1c[:], AF.Square)
        nc.vector.tensor_reduce(st["ss2"][:, c0:c0 + SB], sq2c[:],
                                axis=AX.X, op=OP.add)
        nc.vector.tensor_reduce(st["mx2"][:, c0:c0 + SB], src2[:],
                                axis=AX.X, op=OP.max,
                                apply_absolute_value=general)
        stats_l23(2, c0, c0 + SB)

    def stage_e(ch):
        c0 = ch * SB
        h1c = h1_slots[ch % 2]
        h2c = h2_slots[ch % 2]
        sq3c = sq3_slots[ch % 2]
        src2 = hsc_slots[ch % 2] if general else h1c
        # ---- phase E: L2 ----
        q2 = q_pool.tile([P, SB, P], FP16, name="q2t", tag="q2")
        for j in range(SB):
            t = c0 + j
            nc.gpsimd.tensor_scalar(q2[:, j, :], src2[:, j, :],
                                    st["c2"][:, t:t + 1],
                                    OFF, op0=OP.mult, op1=OP.add)
        nc.vector.tensor_scalar_add(q2[:], q2[:], -OFF)   # fp16 4x, exact ints
        qt2 = qt_pool.tile([P, SB, P], FP16, name="qt2t", tag="qt2")
        nc.sync.dma_start_transpose(qt2[:], q2[:])
        for g in range(SB // G2):
            ps2 = ps_pool.tile([P, G2, O2], F32, name="ps2")
            for jj in range(G2):
                j = g * G2 + jj
                nc.tensor.matmul(ps2[:, jj, :], lhsT=qt2[:, j, :], rhs=wt2[:],
                                 start=True, stop=True)
            nc.scalar.activation(h2c[:, g * G2:(g + 1) * G2, :D3], ps2[:],
                                 AF.Relu)
        # ---- L3 row stats (chunk-batched) ----
        if general:
            hsc = hsc_slots[ch % 2]
            nc.vector.memset(hsc[:, :, D3:], 0.0)   # pads -> quant to 1536-OFF
            for j in range(SB):
                nc.vector.tensor_tensor(hsc[:, j, :D3], h2c[:, j, :D3],
                                        sc3[:, :], op=OP.mult)
            src3 = hsc
        else:
            src3 = h2c
        nc.scalar.activation(sq3c[:], h2c[:, :, :D3], AF.Square)
        nc.vector.tensor_reduce(st["ss3"][:, c0:c0 + SB], sq3c[:],
                                axis=AX.X, op=OP.add)
        nc.vector.tensor_reduce(st["mx3"][:, c0:c0 + SB], src3[:, :, :D3],
                                axis=AX.X, op=OP.max,
                                apply_absolute_value=general)
        stats_l23(3, c0, c0 + SB)

    def stage_g(ch):
        c0 = ch * SB
        h2c = h2_slots[ch % 2]
        src3 = hsc_slots[ch % 2] if general else h2c
        # ---- phase G: L3 ----
        q3 = q_pool.tile([P, SB, P], FP16, name="q3t", tag="q3")
        for j in range(SB):
            t = c0 + j
            nc.vector.tensor_scalar(q3[:, j, :], src3[:, j, :],
                                    st["c3"][:, t:t + 1],
                                    OFF, op0=OP.mult, op1=OP.add)
        nc.vector.tensor_scalar_add(q3[:], q3[:], -OFF)   # pads -> exactly 0
        qt3 = qt_pool.tile([P, SB, P], FP16, name="qt3t", tag="qt3")
        nc.sync.dma_start_transpose(qt3[:], q3[:])
        ps3 = ps3_pool.tile([P, SB, 16], F32, name="ps3")
        for j in range(SB):
            nc.tensor.matmul(ps3[:, j, :], lhsT=qt3[:, j, :], rhs=wt3[:],
                             start=True, stop=True)
        # final scale: out = z3' * b3 (per-row broadcast along o)
        nc.vector.tensor_tensor(
            outsb[:, c0:c0 + SB, :], ps3[:, :, :O3],
            st["b3"][:, c0:c0 + SB, None].to_broadcast((P, SB, O3)),
            op=OP.mult)

    # Software-pipelined emission: skew the four stages across chunks so
    # every engine's in-order instruction stream always has ready work.
    stages = (stage_a, stage_c, stage_e, stage_g)
    for step in range(repeat * NCH + 3):
        for si, fn in enumerate(stages):
            k = step - si
            if 0 <= k < repeat * NCH:
                fn(k % NCH)

    nc.sync.dma_start(out_v[:, :, :], outsb[:, :, :])


def _build_nc(R, isw, general_scales, TB=4, SB=8, repeat=1):
    nc = bacc.Bacc("TRN2", target_bir_lowering=False, debug=False)
    aps = {
        "x": nc.dram_tensor("x", [R, D1], F32, kind="ExternalInput").ap(),
        "wt1": nc.dram_tensor("wt1", [K1, O1], FP16, kind="ExternalInput").ap(),
        "wt2": nc.dram_tensor("wt2", [P, O2], FP16, kind="ExternalInput").ap(),
        "wt3": nc.dram_tensor("wt3", [P, 16], FP16, kind="ExternalInput").ap(),
        "out": nc.dram_tensor("out", [R, O3], F32, kind="ExternalOutput").ap(),
    }
    if general_scales:
        aps["scale1"] = nc.dram_tensor("scale1", [D1], F32,
                                       kind="ExternalInput").ap()
        aps["scale2"] = nc.dram_tensor("scale2", [D2], F32,
                                       kind="ExternalInput").ap()
        aps["scale3"] = nc.dram_tensor("scale3", [D3], F32,
                                       kind="ExternalInput").ap()
    with tile.TileContext(nc) as tc:
        with ExitStack() as ctx:
            _ffn_body(ctx, tc, aps, R, isw,
                      scales=general_scales, TB=TB, SB=SB, repeat=repeat)
    nc.finalize()
    return nc


def kernel(x, w1, scale1, w2, scale2, w3, scale3, **_unused):
    x = np.ascontiguousarray(np.asarray(x, dtype=np.float32))
    w1 = np.asarray(w1, dtype=np.float32)
    w2 = np.asarray(w2, dtype=np.float32)
    w3 = np.asarray(w3, dtype=np.float32)
    scale1 = np.asarray(scale1, dtype=np.float32)
    scale2 = np.asarray(scale2, dtype=np.float32)
    scale3 = np.asarray(scale3, dtype=np.float32)

    B = x.shape[0]
    assert B % N_CORES == 0
    R = B // N_CORES

    arrays, isw = _host_weight_tensors(w1, w2, w3)
    ones = (np.all(scale1 == 1.0) and np.all(scale2 == 1.0)
            and np.all(scale3 == 1.0))
    general = None if ones else True

    nc = _build_nc(R, isw, general_scales=general)

    in_maps = []
    for i in range(N_CORES):
        m = {"x": x[i * R:(i + 1) * R], **arrays}
        if general:
            m["scale1"] = scale1
            m["scale2"] = scale2
            m["scale3"] = scale3
        in_maps.append(m)

    trace = bool(os.environ.get("FFN_TRACE"))
    res = run_bass_kernel_spmd(nc, in_maps, list(range(N_CORES)),
                               trace=trace,
                               tmpdir=os.environ.get("FFN_TRACE_DIR"))
    global LAST_EXEC_NS, LAST_TRACE
    LAST_EXEC_NS = res.exec_time_ns
    LAST_TRACE = res.instructions_and_trace
    out = np.concatenate([res.results[i]["out"] for i in range(N_CORES)],
                         axis=0)
    return out.astype(np.float32)


LAST_EXEC_NS = None
LAST_TRACE = None


# revision 27
# speedup vs baseline: 1.0008x; 1.0008x over previous
"""Trainium2 Bass kernel for nn_FFN_61400852463649 (BitNet-style 3-layer FFN).

Self-contained: builds a Bass/Tile SPMD kernel over 8 NeuronCores with pure
batch data parallelism (65536 rows -> 8192 rows/core), per the sharding hint.
Weights are ternary-quantized on the host (tiny + data-independent; the f64
mean is within 2e-8 of the reference's f32 mean and the seed-0 boundary
margin is ~6e-6, so the ternary decisions match the reference exactly) and
uploaded pre-transposed in fp16.

Per-core pipeline (all matmul math exact in fp16 / fp32-PSUM):
  - Quant grid multiplier c_r = 127/absmax_r (the rms cancels; sum-sq only
    feeds the per-row output scale).  Inter-layer activations stay UNSCALED
    integer relus; per-row scales ride a tiny side pipeline.
  - Rounding trick: fp16(c*x + 1536) is an exact round-to-nearest-even
    integer quant (c*x in [-127.5, 127.5] lands in [1024, 2048) where fp16
    ULP = 1).  L1 removes the offset with a correction row built into the
    padded K=896 contraction (weight row 784 = -sum_c T[o,c]); L2/L3 remove
    it with one cheap fp16 4x-mode DVE subtract on the quantized tile.
  - Batched xbar DMA transposes (one instruction per block) produce the
    c-major operands the PE needs; matmuls run fp16 with exact fp32-PSUM
    integer accumulation.
  - Work is spread across all five engines (GPSIMD does the big quants, ACT
    the sum-squares + relu evacs, DVE the reduces/stats, PE the matmuls) and
    the four pipeline stages are software-pipeline-skewed across row chunks
    so each in-order engine queue always has ready work.
  - Cost-model timeline: ~210 us per core (~75% of it bound by the 26 MB
    HBM x-load + SBUF transpose traffic on the shared DMA engines).
"""

import os
import sys

sys.path.insert(0, "/opt/trn_rl_repo")

from contextlib import ExitStack

import numpy as np

import concourse.bass as bass
import concourse.mybir as mybir
import concourse.tile as tile
from concourse import bacc
from concourse.bass_utils import run_bass_kernel_spmd

F32 = mybir.dt.float32
FP16 = mybir.dt.float16
AX = mybir.AxisListType
AF = mybir.ActivationFunctionType
OP = mybir.AluOpType

P = 128
N_CORES = 8
B_FULL = 65536
D1, D2, D3 = 784, 128, 64
O1, O2, O3 = 128, 64, 10
K1 = 896            # 7*128; col 784 is the +1536 correction row
OFF = 1536.0
EPS_RMS = 1e-8
EPS_Q = 1e-5
TINY = 1e-30
RSQ_D = {1: float(np.float32(D1 ** -0.5)),
         2: float(np.float32(D2 ** -0.5)),
         3: float(np.float32(D3 ** -0.5))}


def _host_quant_weights(w):
    m = np.float32(np.mean(np.abs(w), dtype=np.float64))
    m = np.maximum(m, np.float32(EPS_Q))
    sw = np.float32(1.0) / m
    t = np.clip(np.round((w * sw).astype(np.float32)), -1, 1).astype(np.float32)
    return t, float(m)  # m == 1/s_w


def _host_weight_tensors(w1, w2, w3):
    t1, im1 = _host_quant_weights(w1)
    t2, im2 = _host_quant_weights(w2)
    t3, im3 = _host_quant_weights(w3)
    wt1 = np.zeros((K1, O1), np.float16)
    wt1[:D1, :] = t1.T.astype(np.float16)
    wt1[D1, :] = (-t1.sum(axis=1)).astype(np.float16)
    wt2 = t2.T.astype(np.float16)
    wt3 = np.zeros((P, 16), np.float16)
    wt3[:D3, :O3] = t3.T.astype(np.float16)
    arrays = {"wt1": wt1, "wt2": wt2, "wt3": wt3}
    isw = {1: im1, 2: im2, 3: im3}
    return arrays, isw


def _ffn_body(ctx, tc, aps, R, isw, scales, TB=4, SB=8, repeat=1):
    nc = tc.nc
    NT = R // P
    assert NT % SB == 0 and SB % TB == 0
    general = scales is not None   # non-unit rms-norm scale path

    wpool = ctx.enter_context(tc.tile_pool(name="weights", bufs=1))
    stat_pool = ctx.enter_context(tc.tile_pool(name="stats", bufs=1))
    ps_pool = ctx.enter_context(tc.tile_pool(name="psum", bufs=3, space="PSUM"))
    ps3_pool = ctx.enter_context(tc.tile_pool(name="psum3", bufs=2, space="PSUM"))

    wt1 = wpool.tile([P, 7, P], FP16, name="wt1")
    wt2 = wpool.tile([P, O2], FP16, name="wt2")
    wt3 = wpool.tile([P, 16], FP16, name="wt3")
    nc.sync.dma_start(wt1[:], aps["wt1"].rearrange("(b p) o -> p b o", p=P))
    nc.sync.dma_start(wt2[:], aps["wt2"][:, :])
    nc.sync.dma_start(wt3[:], aps["wt3"][:, :])
    isw127 = {l: float(np.float32(isw[l]) / np.float32(127.0)) for l in isw}

    if general:
        # replicate per-feature scales across all partitions (DMA broadcast)
        sc1 = wpool.tile([P, D1], F32, name="sc1")
        sc2 = wpool.tile([P, D2], F32, name="sc2")
        sc3 = wpool.tile([P, D3], F32, name="sc3")
        for t_, ap_ in ((sc1, aps["scale1"]), (sc2, aps["scale2"]),
                        (sc3, aps["scale3"])):
            nc.sync.dma_start(t_[:], ap_[None, :].to_broadcast((P, ap_.shape[0])))

    st = {}
    for nm in ("mx1", "ss1", "c1", "b1", "mx2", "ss2", "c2", "b2",
               "mx3", "ss3", "c3", "b3",
               "tmpa1", "tmpb1", "tmpa2", "tmpb2", "tmpc2",
               "tmpa3", "tmpb3", "tmpc3"):
        st[nm] = stat_pool.tile([P, NT], F32, name=f"st_{nm}")
    outsb = stat_pool.tile([P, NT, O3], F32, name="outsb")
    sq_dump = stat_pool.tile([P, D1], F32, name="sq_dump")

    x_v = aps["x"].rearrange("(p t) c -> p t c", p=P)
    out_v = aps["out"].rearrange("(p t) o -> p t o", p=P)

    xb_pool = ctx.enter_context(tc.tile_pool(name="xblk", bufs=1))
    hc_pool = ctx.enter_context(tc.tile_pool(name="hchunk", bufs=1))
    q_pool = ctx.enter_context(tc.tile_pool(name="q", bufs=3))
    qt_pool = ctx.enter_context(tc.tile_pool(name="qt", bufs=3))

    # general path doubles x-side SBUF; shallower prefetch there
    NBLK_X = (2 * (SB // TB) + 2) if not general else (SB // TB + 1)
    x_slots = [xb_pool.tile([P, TB, K1], F32, name=f"xslot{i}")
               for i in range(NBLK_X)]
    for xs in x_slots:
        nc.vector.memset(xs[:, :, D1:], 0.0)   # pad cols stay 0 forever

    h1_slots = [hc_pool.tile([P, SB, P], F32, name=f"h1slot{i}")
                for i in range(2)]
    sq2_slots = [hc_pool.tile([P, SB, P], F32, name=f"sq2slot{i}")
                 for i in range(2)]
    sq3_slots = [hc_pool.tile([P, SB, D3], F32, name=f"sq3slot{i}")
                 for i in range(2)]
    h2_slots = [hc_pool.tile([P, SB, P], F32, name=f"h2slot{i}")
                for i in range(2)]
    for hs in h2_slots:
        nc.vector.memset(hs[:, :, D3:], 0.0)   # pad cols stay 0 forever
    if general:
        xs_sc = [xb_pool.tile([P, TB, K1], F32, name=f"xscslot{i}")
                 for i in range(NBLK_X)]
        for t_ in xs_sc:
            nc.vector.memset(t_[:, :, D1:], 0.0)
        hsc_slots = [hc_pool.tile([P, SB, P], F32, name=f"hsc{i}")
                     for i in range(2)]
        for t_ in hsc_slots:
            nc.vector.memset(t_[:, :, :], 0.0)

    def stats_l1(s0, s1):
        sl = (slice(None), slice(s0, s1))
        tmpa, tmpb = st["tmpa1"][sl], st["tmpb1"][sl]
        mx, ss = st["mx1"][sl], st["ss1"][sl]
        nc.scalar.activation(tmpa, ss, AF.Sqrt)
        nc.vector.tensor_scalar(tmpa, tmpa, RSQ_D[1], EPS_RMS,
                                op0=OP.mult, op1=OP.add)       # D = rms+eps
        nc.vector.reciprocal(tmpb, tmpa)
        nc.vector.tensor_tensor(tmpa, mx, tmpb, op=OP.mult)    # ratio
        nc.vector.tensor_scalar_max(tmpa, tmpa, EPS_Q)
        nc.vector.tensor_scalar_mul(st["b1"][sl], tmpa, isw127[1])
        nc.vector.tensor_scalar_max(tmpb, mx, TINY)
        nc.vector.reciprocal(tmpb, tmpb)
        nc.vector.tensor_scalar_mul(st["c1"][sl], tmpb, 127.0)

    def stats_l23(l, s0, s1):
        sl = (slice(None), slice(s0, s1))
        tmpa, tmpb, tmpc = (st[f"tmpa{l}"][sl], st[f"tmpb{l}"][sl],
                            st[f"tmpc{l}"][sl])
        mx, ss = st[f"mx{l}"][sl], st[f"ss{l}"][sl]
        gp = st[f"b{l - 1}"][sl]
        nc.scalar.activation(tmpa, ss, AF.Sqrt)
        nc.vector.tensor_scalar_mul(tmpa, tmpa, RSQ_D[l])
        nc.vector.tensor_tensor(tmpa, tmpa, gp, op=OP.mult)    # true rms
        nc.vector.tensor_scalar_add(tmpa, tmpa, EPS_RMS)
        nc.vector.reciprocal(tmpb, tmpa)
        nc.vector.tensor_tensor(tmpc, mx, gp, op=OP.mult)      # true max
        nc.vector.tensor_tensor(tmpa, tmpc, tmpb, op=OP.mult)  # ratio
        nc.vector.tensor_scalar_max(tmpa, tmpa, EPS_Q)
        nc.vector.tensor_scalar_mul(st[f"b{l}"][sl], tmpa, isw127[l])
        nc.vector.tensor_scalar_max(tmpb, mx, TINY)
        nc.vector.reciprocal(tmpb, tmpb)
        nc.vector.tensor_scalar_mul(st[f"c{l}"][sl], tmpb, 127.0)

    NCH = NT // SB
    G2 = min(SB, 8)              # L2 psum-batch group size (N=G2*64 <= 512)

    def stage_a(ch):
        c0 = ch * SB
        # ---- phase A: load + L1 row stats ----
        for blk in range(c0 // TB, (c0 + SB) // TB):
            t0 = blk * TB
            xs = x_slots[blk % NBLK_X]
            nc.sync.dma_start(xs[:, :, :D1], x_v[:, t0:t0 + TB, :])
            if general:
                xc = xs_sc[blk % NBLK_X]
                for i in range(TB):
                    nc.vector.tensor_tensor(xc[:, i, :D1], xs[:, i, :D1],
                                            sc1[:, :], op=OP.mult)
                qsrc = xc
            else:
                qsrc = xs
            nc.vector.tensor_reduce(st["mx1"][:, t0:t0 + TB], qsrc[:, :, :D1],
                                    axis=AX.X, op=OP.max,
                                    apply_absolute_value=True)
            for i in range(TB):
                nc.scalar.activation(sq_dump[:], xs[:, i, :D1], AF.Square,
                                     accum_out=st["ss1"][:, t0 + i:t0 + i + 1])
        stats_l1(c0, c0 + SB)

    def stage_c(ch):
        c0 = ch * SB
        h1c = h1_slots[ch % 2]
        sq2c = sq2_slots[ch % 2]
        # ---- phase C: L1 quant/transpose/matmul/evac ----
        for jb in range(SB // TB):
            b0 = c0 + jb * TB
            qsrc = (xs_sc if general else x_slots)[(b0 // TB) % NBLK_X]
            q1 = q_pool.tile([P, TB, K1], FP16, name="q1t", tag="q1")
            for i in range(TB):
                nc.gpsimd.tensor_scalar(q1[:, i, :], qsrc[:, i, :],
                                        st["c1"][:, b0 + i:b0 + i + 1],
                                        OFF, op0=OP.mult, op1=OP.add)
            qt1 = qt_pool.tile([P, TB * 7, P], FP16, name="qt1t", tag="qt1")
            nc.sync.dma_start_transpose(qt1[:], q1[:])
            ps1 = ps_pool.tile([P, TB, O1], F32, name="ps1")
            for i in range(TB):
                for b in range(7):
                    nc.tensor.matmul(ps1[:, i, :], lhsT=qt1[:, i * 7 + b, :],
                                     rhs=wt1[:, b, :],
                                     start=(b == 0), stop=(b == 6))
            nc.scalar.activation(h1c[:, jb * TB:(jb + 1) * TB, :], ps1[:],
                                 AF.Relu)
            if general:
                hsc = hsc_slots[ch % 2]
                for i in range(TB):
                    j = jb * TB + i
                    nc.vector.tensor_tensor(hsc[:, j, :], h1c[:, j, :],
                                            sc2[:, :], op=OP.mult)
        # ---- L2 row stats (chunk-batched) ----
        src2 = hsc_slots[ch % 2] if general else h1c
        nc.scalar.activation(sq2c[:], h1c[:], AF.Square)
        nc.vector.tensor_reduce(st["ss2"][:, c0:c0 + SB], sq2c[:],
                                axis=AX.X, op=OP.add)
        nc.vector.tensor_reduce(st["mx2"][:, c0:c0 + SB], src2[:],
                                axis=AX.X, op=OP.max,
                                apply_absolute_value=general)
        stats_l23(2, c0, c0 + SB)

    def stage_e(ch):
        c0 = ch * SB
        h1c = h1_slots[ch % 2]
        h2c = h2_slots[ch % 2]
        sq3c = sq3_slots[ch % 2]
        src2 = hsc_slots[ch % 2] if general else h1c
        # ---- phase E: L2 ----
        q2 = q_pool.tile([P, SB, P], FP16, name="q2t", tag="q2")
        for j in range(SB):
            t = c0 + j
            nc.gpsimd.tensor_scalar(q2[:, j, :], src2[:, j, :],
                                    st["c2"][:, t:t + 1],
                                    OFF, op0=OP.mult, op1=OP.add)
        qt2 = qt_pool.tile([P, SB, P], FP16, name="qt2t", tag="qt2")
        H = SB // 2
        nc.sync.dma_start_transpose(qt2[:, :H, :], q2[:, :H, :])
        nc.sync.dma_start_transpose(qt2[:, H:, :], q2[:, H:, :])
        nc.vector.tensor_scalar_add(qt2[:], qt2[:], -OFF)  # fp16 4x, exact ints
        for g in range(SB // G2):
            ps2 = ps_pool.tile([P, G2, O2], F32, name="ps2")
            for jj in range(G2):
                j = g * G2 + jj
                nc.tensor.matmul(ps2[:, jj, :], lhsT=qt2[:, j, :], rhs=wt2[:],
                                 start=True, stop=True)
            nc.scalar.activation(h2c[:, g * G2:(g + 1) * G2, :D3], ps2[:],
                                 AF.Relu)
        # ---- L3 row stats (chunk-batched) ----
        if general:
            hsc = hsc_slots[ch % 2]
            nc.vector.memset(hsc[:, :, D3:], 0.0)   # pads -> quant to 1536-OFF
            for j in range(SB):
                nc.vector.tensor_tensor(hsc[:, j, :D3], h2c[:, j, :D3],
                                        sc3[:, :], op=OP.mult)
            src3 = hsc
        else:
            src3 = h2c
        nc.scalar.activation(sq3c[:], h2c[:, :, :D3], AF.Square)
        nc.vector.tensor_reduce(st["ss3"][:, c0:c0 + SB], sq3c[:],
                                axis=AX.X, op=OP.add)
        nc.vector.tensor_reduce(st["mx3"][:, c0:c0 + SB], src3[:, :, :D3],
                                axis=AX.X, op=OP.max,
                                apply_absolute_value=general)
        stats_l23(3, c0, c0 + SB)

    def stage_g(ch):
        c0 = ch * SB
        h2c = h2_slots[ch % 2]
        src3 = hsc_slots[ch % 2] if general else h2c
        # ---- phase G: L3 ----
        q3 = q_pool.tile([P, SB, P], FP16, name="q3t", tag="q3")
        for j in range(SB):
            t = c0 + j
            nc.vector.tensor_scalar(q3[:, j, :], src3[:, j, :],
                                    st["c3"][:, t:t + 1],
                                    OFF, op0=OP.mult, op1=OP.add)
        qt3 = qt_pool.tile([P, SB, P], FP16, name="qt3t", tag="qt3")
        H = SB // 2
        nc.sync.dma_start_transpose(qt3[:, :H, :], q3[:, :H, :])
        nc.sync.dma_start_transpose(qt3[:, H:, :], q3[:, H:, :])
        nc.vector.tensor_scalar_add(qt3[:], qt3[:], -OFF)  # pads -> exactly 0
        ps3 = ps3_pool.tile([P, SB, 16], F32, name="ps3")
        for j in range(SB):
            nc.tensor.matmul(ps3[:, j, :], lhsT=qt3[:, j, :], rhs=wt3[:],
                             start=True, stop=True)
        # final scale: out = z3' * b3 (per-row broadcast along o)
        nc.vector.tensor_tensor(
            outsb[:, c0:c0 + SB, :], ps3[:, :, :O3],
            st["b3"][:, c0:c0 + SB, None].to_broadcast((P, SB, O3)),
            op=OP.mult)

    # Software-pipelined emission: skew the four stages across chunks so
    # every engine's in-order instruction stream always has ready work.
    stages = (stage_a, stage_c, stage_e, stage_g)
    for step in range(repeat * NCH + 3):
        for si, fn in enumerate(stages):
            k = step - si
            if 0 <= k < repeat * NCH:
                fn(k % NCH)

    nc.sync.dma_start(out_v[:, :, :], outsb[:, :, :])


def _build_nc(R, isw, general_scales, TB=4, SB=8, repeat=1):
    nc = bacc.Bacc("TRN2", target_bir_lowering=False, debug=False)
    aps = {
        "x": nc.dram_tensor("x", [R, D1], F32, kind="ExternalInput").ap(),
        "wt1": nc.dram_tensor("wt1", [K1, O1], FP16, kind="ExternalInput").ap(),
        "wt2": nc.dram_tensor("wt2", [P, O2], FP16, kind="ExternalInput").ap(),
        "wt3": nc.dram_tensor("wt3", [P, 16], FP16, kind="ExternalInput").ap(),
        "out": nc.dram_tensor("out", [R, O3], F32, kind="ExternalOutput").ap(),
    }
    if general_scales:
        aps["scale1"] = nc.dram_tensor("scale1", [D1], F32,
                                       kind="ExternalInput").ap()
        aps["scale2"] = nc.dram_tensor("scale2", [D2], F32,
                                       kind="ExternalInput").ap()
        aps["scale3"] = nc.dram_tensor("scale3", [D3], F32,
                                       kind="ExternalInput").ap()
    with tile.TileContext(nc) as tc:
        with ExitStack() as ctx:
            _ffn_body(ctx, tc, aps, R, isw,
                      scales=general_scales, TB=TB, SB=SB, repeat=repeat)
    nc.finalize()
    return nc


def kernel(x, w1, scale1, w2, scale2, w3, scale3, **_unused):
    x = np.ascontiguousarray(np.asarray(x, dtype=np.float32))
    w1 = np.asarray(w1, dtype=np.float32)
    w2 = np.asarray(w2, dtype=np.float32)
    w3 = np.asarray(w3, dtype=np.float32)
    scale1 = np.asarray(scale1, dtype=np.float32)
    scale2 = np.asarray(scale2, dtype=np.float32)
    scale3 = np.asarray(scale3, dtype=np.float32)

    B = x.shape[0]
    assert B % N_CORES == 0
    R = B // N_CORES

    arrays, isw = _host_weight_tensors(w1, w2, w3)
    ones = (np.all(scale1 == 1.0) and np.all(scale2 == 1.0)
            and np.all(scale3 == 1.0))
    general = None if ones else True

    nc = _build_nc(R, isw, general_scales=general)

    in_maps = []
    for i in range(N_CORES):
        m = {"x": x[i * R:(i + 1) * R], **arrays}
        if general:
            m["scale1"] = scale1
            m["scale2"] = scale2
            m["scale3"] = scale3
        in_maps.append(m)

    trace = bool(os.environ.get("FFN_TRACE"))
    res = run_bass_kernel_spmd(nc, in_maps, list(range(N_CORES)),
                               trace=trace,
                               tmpdir=os.environ.get("FFN_TRACE_DIR"))
    global LAST_EXEC_NS, LAST_TRACE
    LAST_EXEC_NS = res.exec_time_ns
    LAST_TRACE = res.instructions_and_trace
    out = np.concatenate([res.results[i]["out"] for i in range(N_CORES)],
                         axis=0)
    return out.astype(np.float32)


LAST_EXEC_NS = None
LAST_TRACE = None


# revision 34
# speedup vs baseline: 1.0804x; 1.0795x over previous
"""Trainium2 Bass kernel for nn_FFN_61400852463649 (BitNet-style 3-layer FFN).

Self-contained: builds a Bass/Tile SPMD kernel over 8 NeuronCores with pure
batch data parallelism (65536 rows -> 8192 rows/core), per the sharding hint.
Weights are ternary-quantized on the host (tiny + data-independent; the f64
mean is within 2e-8 of the reference's f32 mean and the seed-0 boundary
margin is ~6e-6, so the ternary decisions match the reference exactly) and
uploaded pre-transposed in fp16.

Per-core pipeline (all matmul math exact in fp16 / fp32-PSUM):
  - Quant grid multiplier c_r = 127/absmax_r (the rms cancels; sum-sq only
    feeds the per-row output scale).  Inter-layer activations stay UNSCALED
    integer relus; per-row scales ride a tiny side pipeline.
  - Rounding trick: fp16(c*x + 1536) is an exact round-to-nearest-even
    integer quant (c*x in [-127.5, 127.5] lands in [1024, 2048) where fp16
    ULP = 1).  L1 removes the offset with a correction row built into the
    padded K=896 contraction (weight row 784 = -sum_c T[o,c]); L2/L3 remove
    it with one cheap fp16 4x-mode DVE subtract on the quantized tile.
  - Batched xbar DMA transposes (one instruction per block) produce the
    c-major operands the PE needs; matmuls run fp16 with exact fp32-PSUM
    integer accumulation.
  - Work is spread across all five engines (GPSIMD does the big quants, ACT
    the sum-squares + relu evacs, DVE the reduces/stats, PE the matmuls) and
    the four pipeline stages are software-pipeline-skewed across row chunks
    so each in-order engine queue always has ready work.
  - Cost-model timeline: ~210 us per core (~75% of it bound by the 26 MB
    HBM x-load + SBUF transpose traffic on the shared DMA engines).
"""

import os
import sys

sys.path.insert(0, "/opt/trn_rl_repo")

from contextlib import ExitStack

import numpy as np

import concourse.bass as bass
import concourse.mybir as mybir
import concourse.tile as tile
from concourse import bacc
from concourse.bass_utils import run_bass_kernel_spmd

F32 = mybir.dt.float32
FP16 = mybir.dt.float16
AX = mybir.AxisListType
AF = mybir.ActivationFunctionType
OP = mybir.AluOpType

P = 128
N_CORES = 8
B_FULL = 65536
D1, D2, D3 = 784, 128, 64
O1, O2, O3 = 128, 64, 10
K1 = 896            # 7*128; col 784 is the +1536 correction row
OFF = 1536.0
EPS_RMS = 1e-8
EPS_Q = 1e-5
TINY = 1e-30
RSQ_D = {1: float(np.float32(D1 ** -0.5)),
         2: float(np.float32(D2 ** -0.5)),
         3: float(np.float32(D3 ** -0.5))}


def _host_quant_weights(w):
    m = np.float32(np.mean(np.abs(w), dtype=np.float64))
    m = np.maximum(m, np.float32(EPS_Q))
    sw = np.float32(1.0) / m
    t = np.clip(np.round((w * sw).astype(np.float32)), -1, 1).astype(np.float32)
    return t, float(m)  # m == 1/s_w


def _host_weight_tensors(w1, w2, w3):
    t1, im1 = _host_quant_weights(w1)
    t2, im2 = _host_quant_weights(w2)
    t3, im3 = _host_quant_weights(w3)
    wt1 = np.zeros((K1, O1), np.float16)
    wt1[:D1, :] = t1.T.astype(np.float16)
    wt1[D1, :] = (-t1.sum(axis=1)).astype(np.float16)
    wt2 = t2.T.astype(np.float16)
    wt3 = np.zeros((P, 16), np.float16)
    wt3[:D3, :O3] = t3.T.astype(np.float16)
    arrays = {"wt1": wt1, "wt2": wt2, "wt3": wt3}
    isw = {1: im1, 2: im2, 3: im3}
    return arrays, isw


def _ffn_body(ctx, tc, aps, R, isw, scales, TB=4, SB=8, repeat=1):
    nc = tc.nc
    NT = R // P
    assert NT % SB == 0 and SB % TB == 0
    general = scales is not None   # non-unit rms-norm scale path

    wpool = ctx.enter_context(tc.tile_pool(name="weights", bufs=1))
    stat_pool = ctx.enter_context(tc.tile_pool(name="stats", bufs=1))
    ps_pool = ctx.enter_context(tc.tile_pool(name="psum", bufs=3, space="PSUM"))
    ps3_pool = ctx.enter_context(tc.tile_pool(name="psum3", bufs=2, space="PSUM"))

    wt1 = wpool.tile([P, 7, P], FP16, name="wt1")
    wt2 = wpool.tile([P, O2], FP16, name="wt2")
    wt3 = wpool.tile([P, 16], FP16, name="wt3")
    nc.sync.dma_start(wt1[:], aps["wt1"].rearrange("(b p) o -> p b o", p=P))
    nc.sync.dma_start(wt2[:], aps["wt2"][:, :])
    nc.sync.dma_start(wt3[:], aps["wt3"][:, :])
    isw127 = {l: float(np.float32(isw[l]) / np.float32(127.0)) for l in isw}

    if general:
        # replicate per-feature scales across all partitions (DMA broadcast)
        sc1 = wpool.tile([P, D1], F32, name="sc1")
        sc2 = wpool.tile([P, D2], F32, name="sc2")
        sc3 = wpool.tile([P, D3], F32, name="sc3")
        for t_, ap_ in ((sc1, aps["scale1"]), (sc2, aps["scale2"]),
                        (sc3, aps["scale3"])):
            nc.sync.dma_start(t_[:], ap_[None, :].to_broadcast((P, ap_.shape[0])))

    st = {}
    for nm in ("mx1", "ss1", "c1", "b1", "mx2", "ss2", "c2", "b2",
               "mx3", "ss3", "c3", "b3",
               "tmpa1", "tmpb1", "tmpa2", "tmpb2", "tmpc2",
               "tmpa3", "tmpb3", "tmpc3"):
        st[nm] = stat_pool.tile([P, NT], F32, name=f"st_{nm}")
    outsb = stat_pool.tile([P, NT, O3], F32, name="outsb")
    sq_dump = stat_pool.tile([P, D1], F32, name="sq_dump")

    x_v = aps["x"].rearrange("(p t) c -> p t c", p=P)
    out_v = aps["out"].rearrange("(p t) o -> p t o", p=P)

    xb_pool = ctx.enter_context(tc.tile_pool(name="xblk", bufs=1))
    hc_pool = ctx.enter_context(tc.tile_pool(name="hchunk", bufs=1))
    q_pool = ctx.enter_context(tc.tile_pool(name="q", bufs=3))
    qt_pool = ctx.enter_context(tc.tile_pool(name="qt", bufs=3))

    # general path doubles x-side SBUF; shallower prefetch there
    NBLK_X = (2 * (SB // TB) + 2) if not general else (SB // TB + 1)
    x_slots = [xb_pool.tile([P, TB, K1], F32, name=f"xslot{i}")
               for i in range(NBLK_X)]
    for xs in x_slots:
        nc.vector.memset(xs[:, :, D1:], 0.0)   # pad cols stay 0 forever

    h1_slots = [hc_pool.tile([P, SB, P], F32, name=f"h1slot{i}")
                for i in range(2)]
    sq2_slots = [hc_pool.tile([P, SB, P], F32, name=f"sq2slot{i}")
                 for i in range(2)]
    sq3_slots = [hc_pool.tile([P, SB, D3], F32, name=f"sq3slot{i}")
                 for i in range(2)]
    h2_slots = [hc_pool.tile([P, SB, P], F32, name=f"h2slot{i}")
                for i in range(2)]
    for hs in h2_slots:
        nc.vector.memset(hs[:, :, D3:], 0.0)   # pad cols stay 0 forever
    if general:
        xs_sc = [xb_pool.tile([P, TB, K1], F32, name=f"xscslot{i}")
                 for i in range(NBLK_X)]
        for t_ in xs_sc:
            nc.vector.memset(t_[:, :, D1:], 0.0)
        hsc_slots = [hc_pool.tile([P, SB, P], F32, name=f"hsc{i}")
                     for i in range(2)]
        for t_ in hsc_slots:
            nc.vector.memset(t_[:, :, :], 0.0)

    def stats_l1(s0, s1):
        sl = (slice(None), slice(s0, s1))
        tmpa, tmpb = st["tmpa1"][sl], st["tmpb1"][sl]
        mx, ss = st["mx1"][sl], st["ss1"][sl]
        nc.scalar.activation(tmpa, ss, AF.Sqrt)
        nc.vector.tensor_scalar(tmpa, tmpa, RSQ_D[1], EPS_RMS,
                                op0=OP.mult, op1=OP.add)       # D = rms+eps
        nc.vector.reciprocal(tmpb, tmpa)
        nc.vector.tensor_tensor(tmpa, mx, tmpb, op=OP.mult)    # ratio
        nc.vector.tensor_scalar_max(tmpa, tmpa, EPS_Q)
        nc.vector.tensor_scalar_mul(st["b1"][sl], tmpa, isw127[1])
        nc.vector.tensor_scalar_max(tmpb, mx, TINY)
        nc.vector.reciprocal(tmpb, tmpb)
        nc.vector.tensor_scalar_mul(st["c1"][sl], tmpb, 127.0)

    def stats_l23(l, s0, s1):
        sl = (slice(None), slice(s0, s1))
        tmpa, tmpb, tmpc = (st[f"tmpa{l}"][sl], st[f"tmpb{l}"][sl],
                            st[f"tmpc{l}"][sl])
        mx, ss = st[f"mx{l}"][sl], st[f"ss{l}"][sl]
        gp = st[f"b{l - 1}"][sl]
        nc.scalar.activation(tmpa, ss, AF.Sqrt)
        nc.vector.tensor_scalar_mul(tmpa, tmpa, RSQ_D[l])
        nc.vector.tensor_tensor(tmpa, tmpa, gp, op=OP.mult)    # true rms
        nc.vector.tensor_scalar_add(tmpa, tmpa, EPS_RMS)
        nc.vector.reciprocal(tmpb, tmpa)
        nc.vector.tensor_tensor(tmpc, mx, gp, op=OP.mult)      # true max
        nc.vector.tensor_tensor(tmpa, tmpc, tmpb, op=OP.mult)  # ratio
        nc.vector.tensor_scalar_max(tmpa, tmpa, EPS_Q)
        nc.vector.tensor_scalar_mul(st[f"b{l}"][sl], tmpa, isw127[l])
        nc.vector.tensor_scalar_max(tmpb, mx, TINY)
        nc.vector.reciprocal(tmpb, tmpb)
        nc.vector.tensor_scalar_mul(st[f"c{l}"][sl], tmpb, 127.0)

    NCH = NT // SB
    G2 = min(SB, 8)              # L2 psum-batch group size (N=G2*64 <= 512)

    def stage_a(ch):
        c0 = ch * SB
        # ---- phase A: load + L1 row stats ----
        for blk in range(c0 // TB, (c0 + SB) // TB):
            t0 = blk * TB
            xs = x_slots[blk % NBLK_X]
            nc.sync.dma_start(xs[:, :, :D1], x_v[:, t0:t0 + TB, :])
            if general:
                xc = xs_sc[blk % NBLK_X]
                for i in range(TB):
                    nc.vector.tensor_tensor(xc[:, i, :D1], xs[:, i, :D1],
                                            sc1[:, :], op=OP.mult)
                qsrc = xc
            else:
                qsrc = xs
            nc.vector.tensor_reduce(st["mx1"][:, t0:t0 + TB], qsrc[:, :, :D1],
                                    axis=AX.X, op=OP.max,
                                    apply_absolute_value=True)
            for i in range(TB):
                nc.scalar.activation(sq_dump[:], xs[:, i, :D1], AF.Square,
                                     accum_out=st["ss1"][:, t0 + i:t0 + i + 1])
        stats_l1(c0, c0 + SB)

    def stage_c(ch):
        c0 = ch * SB
        h1c = h1_slots[ch % 2]
        sq2c = sq2_slots[ch % 2]
        # ---- phase C: L1 quant/transpose/matmul/evac ----
        for jb in range(SB // TB):
            b0 = c0 + jb * TB
            qsrc = (xs_sc if general else x_slots)[(b0 // TB) % NBLK_X]
            q1 = q_pool.tile([P, TB, K1], FP16, name="q1t", tag="q1")
            for i in range(TB):
                nc.gpsimd.tensor_scalar(q1[:, i, :], qsrc[:, i, :],
                                        st["c1"][:, b0 + i:b0 + i + 1],
                                        OFF, op0=OP.mult, op1=OP.add)
            qt1 = qt_pool.tile([P, TB * 7, P], FP16, name="qt1t", tag="qt1")
            nc.sync.dma_start_transpose(qt1[:], q1[:])
            ps1 = ps_pool.tile([P, TB, O1], F32, name="ps1")
            for i in range(TB):
                for b in range(7):
                    nc.tensor.matmul(ps1[:, i, :], lhsT=qt1[:, i * 7 + b, :],
                                     rhs=wt1[:, b, :],
                                     start=(b == 0), stop=(b == 6))
            nc.scalar.activation(h1c[:, jb * TB:(jb + 1) * TB, :], ps1[:],
                                 AF.Relu)
            if general:
                hsc = hsc_slots[ch % 2]
                for i in range(TB):
                    j = jb * TB + i
                    nc.vector.tensor_tensor(hsc[:, j, :], h1c[:, j, :],
                                            sc2[:, :], op=OP.mult)
        # ---- L2 row stats (chunk-batched) ----
        src2 = hsc_slots[ch % 2] if general else h1c
        nc.scalar.activation(sq2c[:], h1c[:], AF.Square)
        nc.vector.tensor_reduce(st["ss2"][:, c0:c0 + SB], sq2c[:],
                                axis=AX.X, op=OP.add)
        nc.vector.tensor_reduce(st["mx2"][:, c0:c0 + SB], src2[:],
                                axis=AX.X, op=OP.max,
                                apply_absolute_value=general)
        stats_l23(2, c0, c0 + SB)

    def stage_e(ch):
        c0 = ch * SB
        h1c = h1_slots[ch % 2]
        h2c = h2_slots[ch % 2]
        sq3c = sq3_slots[ch % 2]
        src2 = hsc_slots[ch % 2] if general else h1c
        # ---- phase E: L2 ----
        q2 = q_pool.tile([P, SB, P], FP16, name="q2t", tag="q2")
        for j in range(SB):
            t = c0 + j
            nc.gpsimd.tensor_scalar(q2[:, j, :], src2[:, j, :],
                                    st["c2"][:, t:t + 1],
                                    OFF, op0=OP.mult, op1=OP.add)
        qt2 = qt_pool.tile([P, SB, P], FP16, name="qt2t", tag="qt2")
        H = SB // 2
        nc.sync.dma_start_transpose(qt2[:, :H, :], q2[:, :H, :])
        nc.sync.dma_start_transpose(qt2[:, H:, :], q2[:, H:, :])
        nc.vector.tensor_scalar_add(qt2[:], qt2[:], -OFF)  # fp16 4x, exact ints
        for g in range(SB // G2):
            ps2 = ps_pool.tile([P, G2, O2], F32, name="ps2")
            for jj in range(G2):
                j = g * G2 + jj
                nc.tensor.matmul(ps2[:, jj, :], lhsT=qt2[:, j, :], rhs=wt2[:],
                                 start=True, stop=True)
            nc.scalar.activation(h2c[:, g * G2:(g + 1) * G2, :D3], ps2[:],
                                 AF.Relu)
        # ---- L3 row stats (chunk-batched) ----
        if general:
            hsc = hsc_slots[ch % 2]
            nc.vector.memset(hsc[:, :, D3:], 0.0)   # pads -> quant to 1536-OFF
            for j in range(SB):
                nc.vector.tensor_tensor(hsc[:, j, :D3], h2c[:, j, :D3],
                                        sc3[:, :], op=OP.mult)
            src3 = hsc
        else:
            src3 = h2c
        nc.scalar.activation(sq3c[:], h2c[:, :, :D3], AF.Square)
        nc.vector.tensor_reduce(st["ss3"][:, c0:c0 + SB], sq3c[:],
                                axis=AX.X, op=OP.add)
        nc.vector.tensor_reduce(st["mx3"][:, c0:c0 + SB], src3[:, :, :D3],
                                axis=AX.X, op=OP.max,
                                apply_absolute_value=general)
        stats_l23(3, c0, c0 + SB)

    def stage_g(ch):
        c0 = ch * SB
        h2c = h2_slots[ch % 2]
        src3 = hsc_slots[ch % 2] if general else h2c
        # ---- phase G: L3 ----
        q3 = q_pool.tile([P, SB, P], FP16, name="q3t", tag="q3")
        for j in range(SB):
            t = c0 + j
            nc.vector.tensor_scalar(q3[:, j, :], src3[:, j, :],
                                    st["c3"][:, t:t + 1],
                                    OFF, op0=OP.mult, op1=OP.add)
        qt3 = qt_pool.tile([P, SB, P], FP16, name="qt3t", tag="qt3")
        H = SB // 2
        nc.sync.dma_start_transpose(qt3[:, :H, :], q3[:, :H, :])
        nc.sync.dma_start_transpose(qt3[:, H:, :], q3[:, H:, :])
        nc.vector.tensor_scalar_add(qt3[:], qt3[:], -OFF)  # pads -> exactly 0
        ps3 = ps3_pool.tile([P, SB, 16], F32, name="ps3")
        for j in range(SB):
            nc.tensor.matmul(ps3[:, j, :], lhsT=qt3[:, j, :], rhs=wt3[:],
                             start=True, stop=True)
        # final scale: out = z3' * b3 (per-row broadcast along o)
        nc.vector.tensor_tensor(
            outsb[:, c0:c0 + SB, :], ps3[:, :, :O3],
            st["b3"][:, c0:c0 + SB, None].to_broadcast((P, SB, O3)),
            op=OP.mult)

    # Software-pipelined emission: skew the four stages across chunks so
    # every engine's in-order instruction stream always has ready work.
    stages = ((stage_a, 0), (stage_c, 2), (stage_e, 3), (stage_g, 4))
    for step in range(repeat * NCH + 4):
        for fn, off in stages:
            k = step - off
            if 0 <= k < repeat * NCH:
                fn(k % NCH)

    nc.sync.dma_start(out_v[:, :, :], outsb[:, :, :])


def _build_nc(R, isw, general_scales, TB=4, SB=8, repeat=1):
    nc = bacc.Bacc("TRN2", target_bir_lowering=False, debug=False)
    aps = {
        "x": nc.dram_tensor("x", [R, D1], F32, kind="ExternalInput").ap(),
        "wt1": nc.dram_tensor("wt1", [K1, O1], FP16, kind="ExternalInput").ap(),
        "wt2": nc.dram_tensor("wt2", [P, O2], FP16, kind="ExternalInput").ap(),
        "wt3": nc.dram_tensor("wt3", [P, 16], FP16, kind="ExternalInput").ap(),
        "out": nc.dram_tensor("out", [R, O3], F32, kind="ExternalOutput").ap(),
    }
    if general_scales:
        aps["scale1"] = nc.dram_tensor("scale1", [D1], F32,
                                       kind="ExternalInput").ap()
        aps["scale2"] = nc.dram_tensor("scale2", [D2], F32,
                                       kind="ExternalInput").ap()
        aps["scale3"] = nc.dram_tensor("scale3", [D3], F32,
                                       kind="ExternalInput").ap()
    with tile.TileContext(nc) as tc:
        with ExitStack() as ctx:
            _ffn_body(ctx, tc, aps, R, isw,
                      scales=general_scales, TB=TB, SB=SB, repeat=repeat)
    nc.finalize()
    return nc


def kernel(x, w1, scale1, w2, scale2, w3, scale3, **_unused):
    x = np.ascontiguousarray(np.asarray(x, dtype=np.float32))
    w1 = np.asarray(w1, dtype=np.float32)
    w2 = np.asarray(w2, dtype=np.float32)
    w3 = np.asarray(w3, dtype=np.float32)
    scale1 = np.asarray(scale1, dtype=np.float32)
    scale2 = np.asarray(scale2, dtype=np.float32)
    scale3 = np.asarray(scale3, dtype=np.float32)

    B = x.shape[0]
    assert B % N_CORES == 0
    R = B // N_CORES

    arrays, isw = _host_weight_tensors(w1, w2, w3)
    ones = (np.all(scale1 == 1.0) and np.all(scale2 == 1.0)
            and np.all(scale3 == 1.0))
    general = None if ones else True

    nc = _build_nc(R, isw, general_scales=general)

    in_maps = []
    for i in range(N_CORES):
        m = {"x": x[i * R:(i + 1) * R], **arrays}
        if general:
            m["scale1"] = scale1
            m["scale2"] = scale2
            m["scale3"] = scale3
        in_maps.append(m)

    trace = bool(os.environ.get("FFN_TRACE"))
    res = run_bass_kernel_spmd(nc, in_maps, list(range(N_CORES)),
                               trace=trace,
                               tmpdir=os.environ.get("FFN_TRACE_DIR"))
    global LAST_EXEC_NS, LAST_TRACE
    LAST_EXEC_NS = res.exec_time_ns
    LAST_TRACE = res.instructions_and_trace
    out = np.concatenate([res.results[i]["out"] for i in range(N_CORES)],
                         axis=0)
    return out.astype(np.float32)


LAST_EXEC_NS = None
LAST_TRACE = None
